# revision 36
# baseline (speedup 1.0000x reference)
# Multi-head attention (N=4, L=2048, D=1024, H=16, DK=64) on 8 NeuronCores.
#
# Sharding: batch x head-half tensor-parallel. Core c = (n, hh) computes the
# full 2048-q attention of batch n for heads [hh*8, hh*8+8), then the partial
# output projection over its 512 e-dims (WO row-sharded). The host sums the
# two partials per batch and adds the bias. No K/V projection is duplicated
# anywhere (pure DP over (n, q-half) computes each batch's K/V proj twice).
#
# Per-core pipeline, engine-balanced (ScalarE exp is the floor: 33.5M
# elements at 1 elem/lane/cycle @1.2GHz):
#   PE     : V/K/Q projections, S^T = KiT^T QiT (contract 64), PV (contract
#            128, M=65 with a ones column for row sums), partial out-proj.
#   ScalarE: exp(S/8) [128,1024] psum->sbuf bf16; V/K evacuations during the
#            startup era before exp begins.
#   DVE    : mask multiply (one broadcast op per k-tile), unnormalized pv
#            evacuation, lazy batched reciprocals, normalize muls, Q evacs.
#   GpSimd : mask multiply for every 4th k-tile (offloads DVE).
#   DMA    : row-sum spill, partition-broadcast of reciprocals (dram round
#            trip), output stores.
# The first attention pass streams directly behind the K projection inside
# the projection era; later passes interleave with remaining projections,
# normalizes and out-proj chunks through a filler queue. PSUM: st 2x2 banks
# + pv 2 + proj/out 2 = 8.
import sys

sys.path.insert(0, "/opt/trn_rl_repo")

import collections
from contextlib import ExitStack

import numpy as np
import ml_dtypes

N, QLEN, KLEN, DMODEL, NHEAD, DK = 4, 2048, 2048, 1024, 16, 64
NCORES = 8
P = 128
HPC = NHEAD // 2  # 8 heads per core
EH = HPC * DK  # 512 e-dims per core
EO = EH // P  # 4 e-tiles (= head pairs)
DO = DMODEL // P  # 8 d-tiles
KO = KLEN // P  # 16 k-tiles
NQQ = 4  # attention q-blocks
QQ = QLEN // NQQ  # 512 q per block
SKEW = 2  # PV trails S/exp/mask by this many k-tiles

_prog_cache = {}


def _build_program():
    import concourse.tile as tile
    from concourse import bacc, mybir

    f32 = mybir.dt.float32
    f32r = mybir.dt.float32r
    bf16 = mybir.dt.bfloat16
    Exp = mybir.ActivationFunctionType.Exp

    nc = bacc.Bacc("TRN2", target_bir_lowering=False, debug=False)

    qT = nc.dram_tensor("qT", (DMODEL, QLEN), f32r, kind="ExternalInput").ap()
    kT = nc.dram_tensor("kT", (DMODEL, KLEN), f32r, kind="ExternalInput").ap()
    vT = nc.dram_tensor("vT", (DMODEL, KLEN), bf16, kind="ExternalInput").ap()
    maskT = nc.dram_tensor("maskT", (KLEN, QLEN), bf16, kind="ExternalInput").ap()
    wq = nc.dram_tensor("wq", (DMODEL, EH), f32r, kind="ExternalInput").ap()
    wk = nc.dram_tensor("wk", (DMODEL, EH), f32r, kind="ExternalInput").ap()
    wv = nc.dram_tensor("wv", (DMODEL, EH), bf16, kind="ExternalInput").ap()
    wo = nc.dram_tensor("wo", (EH, DMODEL), bf16, kind="ExternalInput").ap()
    out = nc.dram_tensor("out", (QLEN, DMODEL), bf16, kind="ExternalOutput").ap()
    rs_dram = nc.dram_tensor("rs_scratch", (NQQ, HPC, QQ), bf16).ap()
    ri_dram = nc.dram_tensor("ri_scratch", (NQQ, HPC, QQ), bf16).ap()

    qT_r = qT.rearrange("(do p) q -> p do q", p=P)
    kT_r = kT.rearrange("(do p) k -> p do k", p=P)
    vT_r = vT.rearrange("(do p) k -> p do k", p=P)
    wq_r = wq.rearrange("(do p) e -> p do e", p=P)
    wk_r = wk.rearrange("(do p) e -> p do e", p=P)
    wv_r = wv.rearrange("(do p) e -> p do e", p=P)
    wo_r = wo.rearrange("(eo p) d -> p eo d", p=P)
    maskT_r = maskT.rearrange("(ko p) q -> p ko q", p=P)

    with tile.TileContext(nc) as tc, ExitStack() as top:
        res = top.enter_context(tc.tile_pool(name="res", bufs=1))
        QiT_s = res.tile([P, EO, QLEN], bf16)  # e=eo*128+p, eo==head pair
        KiT_s = res.tile([P, EO, KLEN], bf16)
        Vi_s = res.tile([P, KO, HPC * 65], bf16)  # k=ko*128+p; col h*65+64 = 1
        headiT_s = res.tile([P, EO, QLEN], bf16)
        wo_s = res.tile([P, EO, DMODEL], bf16)

        gps = top.enter_context(tc.tile_pool(name="gpsum", bufs=2, space="PSUM"))
        sps = top.enter_context(tc.tile_pool(name="spsum", bufs=2, space="PSUM"))
        pvs = top.enter_context(tc.tile_pool(name="pvsum", bufs=1, space="PSUM"))
        ptp = top.enter_context(tc.tile_pool(name="ptile", bufs=SKEW + 4))
        erl = top.enter_context(tc.tile_pool(name="early", bufs=1))

        mask_tiles = {}
        q_stripes = {}
        wq_tiles = {}
        q_psum = {}
        q_done = set()
        filler = collections.deque()

        def drain(n):
            for _ in range(n):
                if not filler:
                    return
                item = filler.popleft()
                if callable(item):
                    item()
                else:  # generator: run one step, put its next step up front
                    try:
                        next(item)
                        filler.appendleft(item)
                    except StopIteration:
                        pass

        def early_loads():
            # q-block-0 inputs, the Q weights (loaded once, shared by every
            # q-block), and the first half of the q-block-0 mask
            qs0 = erl.tile([P, DO, QQ], f32r, tag="qT0")
            for do in range(DO):
                nc.sync.dma_start(qs0[:, do], qT_r[:, do, 0:QQ])
            wcol = erl.tile([P, DO, P], f32r, tag="wq0", name="wq0")
            nc.sync.dma_start(wcol[:], wq_r[:, :, 0:P])
            wq_tiles[0] = wcol
            mk0 = erl.tile([P, KO, QQ], bf16, tag="mask0")
            for ko in range(KO):
                nc.sync.dma_start(mk0[:, ko], maskT_r[:, ko, 0:QQ])
            q_stripes[0] = qs0
            mask_tiles[0] = mk0

        def q_proj(qq, eo, halves=(0, 1)):
            if 0 in halves:
                self_pt = gps.tile([P, 512], f32, tag="gps", name=f"psq{qq}_{eo}")
                q_psum[(qq, eo)] = self_pt
            pt = q_psum[(qq, eo)]
            wcol = wq_tiles[eo]
            qs = q_stripes[qq]
            for h in halves:
                for do in range(h * DO // 2, (h + 1) * DO // 2):
                    nc.tensor.matmul(
                        pt[:],
                        lhsT=wcol[:, do],
                        rhs=qs[:, do],
                        start=(do == 0),
                        stop=(do == DO - 1),
                    )
            if 1 in halves:
                nc.vector.tensor_copy(
                    out=QiT_s[:, eo, qq * QQ : (qq + 1) * QQ], in_=pt[:]
                )
                del q_psum[(qq, eo)]
                q_done.add((qq, eo))

        class AttnPass:
            def __init__(self, qq, hp):
                self.qq, self.hp = qq, hp
                self.mk = mask_tiles[qq]
                self.pv = None
                self.ptq = {}
                self.next_ko = 0

            def emit_pv(self, ko):
                if self.pv is None:
                    # allocated lazily so the pool sees this AFTER the
                    # previous pass's trailing reads (cross-pass pipelining)
                    self.pv = [
                        pvs.tile(
                            [65, QQ], f32, tag=f"pv{i}",
                            name=f"pv{i}_{self.qq}_{self.hp}",
                        )
                        for i in range(2)
                    ]
                pt = self.ptq.pop(ko)
                hp = self.hp
                for i in range(2):
                    nc.tensor.matmul(
                        self.pv[i][:],
                        lhsT=Vi_s[:, ko, (2 * hp + i) * 65 : (2 * hp + i + 1) * 65],
                        rhs=pt[:, i * QQ : (i + 1) * QQ],
                        start=(ko == 0),
                        stop=(ko == KO - 1),
                        skip_group_check=True,
                    )

            def steps(self, ko_end, do_drain=True):
                qq, hp = self.qq, self.hp
                for ko in range(self.next_ko, ko_end):
                    if ko >= SKEW:
                        self.emit_pv(ko - SKEW)
                    st = sps.tile([P, 2 * QQ], f32, tag="st", name=f"st{qq}_{hp}_{ko}")
                    for i in range(2):
                        p0 = 64 * i
                        nc.tensor.matmul(
                            st[:, i * QQ : (i + 1) * QQ],
                            lhsT=KiT_s[p0 : p0 + 64, hp, ko * P : (ko + 1) * P],
                            rhs=QiT_s[p0 : p0 + 64, hp, qq * QQ : (qq + 1) * QQ],
                            start=True,
                            stop=True,
                        )
                    pt = ptp.tile([P, 2 * QQ], bf16, tag="pt", name=f"pt{qq}_{hp}_{ko}")
                    nc.scalar.activation(out=pt[:], in_=st[:], func=Exp, scale=0.125)
                    if ko % 4 == 3:  # give GpSimd a share of the mask work
                        for i in range(2):
                            nc.gpsimd.tensor_mul(
                                out=pt[:, i * QQ : (i + 1) * QQ],
                                in0=pt[:, i * QQ : (i + 1) * QQ],
                                in1=self.mk[:, ko],
                            )
                    else:
                        nc.vector.tensor_mul(
                            out=pt[:].rearrange("p (i q) -> p i q", i=2),
                            in0=pt[:].rearrange("p (i q) -> p i q", i=2),
                            in1=self.mk[:, ko, None, :].to_broadcast([P, 2, QQ]),
                        )
                    self.ptq[ko] = pt
                    if do_drain and ko % 2 == 1:
                        drain(1)
                self.next_ko = ko_end

            def finish(self):
                qq, hp = self.qq, self.hp
                for ko in range(KO - SKEW, KO):
                    self.emit_pv(ko)
                # evacuate unnormalized heads; spill row sums for the lazy
                # batched normalize.
                rssb = rsp.tile([1, 2, QQ], bf16, tag="rssb", name=f"rssb{qq}_{hp}")
                for i in range(2):
                    nc.vector.tensor_copy(
                        out=headiT_s[64 * i : 64 * i + 64, hp, qq * QQ : (qq + 1) * QQ],
                        in_=self.pv[i][0:64, :],
                    )
                    nc.scalar.copy(out=rssb[:, i, :], in_=self.pv[i][64:65, :])
                nc.sync.dma_start(rs_dram[qq, 2 * hp : 2 * hp + 2], rssb[0:1])

        # ---------- projection era: V + K stripes, pass (0,0) streamed ----
        p00 = None
        early_loads()
        with ExitStack() as ph:
            vsp = ph.enter_context(tc.tile_pool(name="vstripe", bufs=2))
            ksp = ph.enter_context(tc.tile_pool(name="kstripe", bufs=2))
            wvp = ph.enter_context(tc.tile_pool(name="wvres", bufs=1))
            wv_s = wvp.tile([P, DO, EH], bf16)
            for do in range(DO):
                nc.sync.dma_start(wv_s[:, do], wv_r[:, do])
            wkp = ph.enter_context(tc.tile_pool(name="wkp", bufs=1))
            wks = {}
            for eo in range(EO):
                wcol = wkp.tile([P, DO, P], f32r, tag=f"wk{eo}", name=f"wk{eo}")
                nc.sync.dma_start(wcol[:], wk_r[:, :, eo * P : (eo + 1) * P])
                wks[eo] = wcol
            nc.vector.memset(Vi_s[:, :, 64::65], 1.0)
            for s in range(4):  # 512-column stripes of kT; 2x256 of vT
                for half in range(2):
                    vss = 2 * s + half
                    vs = vsp.tile([P, DO, 256], bf16, tag="v", name=f"vT{vss}")
                    for do in range(DO):
                        nc.sync.dma_start(
                            vs[:, do], vT_r[:, do, vss * 256 : (vss + 1) * 256]
                        )
                    for t in range(2):
                        ko = vss * 2 + t
                        pt = gps.tile([P, 512], f32, tag="gps", name=f"psv{ko}")
                        for do in range(DO):
                            nc.tensor.matmul(
                                pt[:],
                                lhsT=vs[:, do, t * P : (t + 1) * P],
                                rhs=wv_s[:, do],
                                start=(do == 0),
                                stop=(do == DO - 1),
                            )
                        nc.scalar.copy(
                            out=Vi_s[:, ko, :].rearrange("p (h j) -> p h j", j=65)[
                                :, :, 0:64
                            ],
                            in_=pt[:].rearrange("p (h j) -> p h j", j=64),
                        )
                ks = ksp.tile([P, DO, 512], f32r, tag="k", name=f"kT{s}")
                for do in range(DO):
                    nc.sync.dma_start(ks[:, do], kT_r[:, do, s * 512 : (s + 1) * 512])
                for eo in range(EO):
                    pt = gps.tile([P, 512], f32, tag="gps", name=f"psk{s}_{eo}")
                    for do in range(DO):
                        nc.tensor.matmul(
                            pt[:],
                            lhsT=wks[eo][:, do],
                            rhs=ks[:, do],
                            start=(do == 0),
                            stop=(do == DO - 1),
                        )
                    if eo % 2 == 0:
                        nc.scalar.copy(
                            out=KiT_s[:, eo, s * 512 : (s + 1) * 512], in_=pt[:]
                        )
                    else:
                        nc.vector.tensor_copy(
                            out=KiT_s[:, eo, s * 512 : (s + 1) * 512], in_=pt[:]
                        )
                # stream the first attention pass right behind the K proj
                if s == 0:
                    q_proj(0, 0)
                    p00 = AttnPass(0, 0)
                    continue
                if s == 1:
                    for eo in range(1, EO):
                        wcol = erl.tile(
                            [P, DO, P], f32r, tag=f"wq{eo}", name=f"wq{eo}"
                        )
                        nc.sync.dma_start(wcol[:], wq_r[:, :, eo * P : (eo + 1) * P])
                        wq_tiles[eo] = wcol
                if s == 3:
                    for eo in range(EO):
                        nc.sync.dma_start(wo_s[:, eo], wo_r[:, eo])
                p00.steps(min(4 * s - 2, KO), do_drain=(s >= 2))

        mkp = top.enter_context(tc.tile_pool(name="mask", bufs=2))
        qsp = top.enter_context(tc.tile_pool(name="qstripe", bufs=2))
        nrm = top.enter_context(tc.tile_pool(name="norm", bufs=2))
        rsp = top.enter_context(tc.tile_pool(name="rsp", bufs=1))

        def load_mask(qq):
            mk = mkp.tile([P, KO, QQ], bf16, tag="mask", name=f"mask{qq}")
            for ko in range(KO):
                nc.sync.dma_start(mk[:, ko], maskT_r[:, ko, qq * QQ : (qq + 1) * QQ])
            mask_tiles[qq] = mk

        def load_qstripe(qq):
            qs = qsp.tile([P, DO, QQ], f32r, tag="qT", name=f"qT{qq}")
            for do in range(DO):
                nc.sync.dma_start(qs[:, do], qT_r[:, do, qq * QQ : (qq + 1) * QQ])
            q_stripes[qq] = qs

        def normalize(qq, hq):
            # 1/rowsum for one head-quad: rowsums were spilled to rs_dram by
            # the two finished passes; DVE reciprocal, spill back, then
            # partition-broadcast DMAs feed the normalize multiply.
            sl = slice(4 * hq, 4 * hq + 4)
            rsall = rsp.tile([4, QQ], bf16, tag="rsall", name=f"rsa{qq}_{hq}")
            nc.sync.dma_start(rsall[:], rs_dram[qq, sl])
            with nc.allow_low_precision(reason="softmax reciprocal in bf16"):
                nc.vector.reciprocal(out=rsall[:], in_=rsall[:])
            yield
            nc.sync.dma_start(ri_dram[qq, sl], rsall[:])
            rr = rsp.tile([P, 2, QQ], bf16, tag="rr", name=f"rr{qq}_{hq}")
            for j in range(2):
                for i in range(2):
                    nc.sync.dma_start(
                        rr[64 * i : 64 * i + 64, j, :],
                        ri_dram[qq, 2 * (2 * hq + j) + i].partition_broadcast(64),
                    )
            yield
            nc.vector.tensor_mul(
                out=headiT_s[:, 2 * hq : 2 * hq + 2, qq * QQ : (qq + 1) * QQ],
                in0=headiT_s[:, 2 * hq : 2 * hq + 2, qq * QQ : (qq + 1) * QQ],
                in1=rr[:],
            )
            yield

        def out_proj(qq):
            for t in range(QQ // P):  # 4 q-tiles of 128
                qt = qq * (QQ // P) + t
                for dc in range(2):
                    pt = gps.tile([P, 512], f32, tag="gps", name=f"pso{qt}_{dc}")
                    for eo in range(EO):
                        nc.tensor.matmul(
                            pt[:],
                            lhsT=headiT_s[:, eo, qt * P : (qt + 1) * P],
                            rhs=wo_s[:, eo, dc * 512 : (dc + 1) * 512],
                            start=(eo == 0),
                            stop=(eo == EO - 1),
                        )
                    ot = nrm.tile([P, 512], bf16, tag="ot", name=f"o{qt}_{dc}")
                    nc.vector.tensor_copy(out=ot[:], in_=pt[:])
                    nc.sync.dma_start(
                        out[qt * P : (qt + 1) * P, dc * 512 : (dc + 1) * 512],
                        ot[:],
                    )
                    yield

        for eo in range(1, EO):
            filler.append(lambda eo=eo: q_proj(0, eo, (0,)))
            filler.append(lambda eo=eo: q_proj(0, eo, (1,)))

        def queue_setup(qq):
            filler.append(lambda: (load_mask(qq), load_qstripe(qq)))
            for eo in range(EO):
                filler.append(lambda eo=eo: q_proj(qq, eo, (0,)))
                filler.append(lambda eo=eo: q_proj(qq, eo, (1,)))

        # ---------- attention passes (software-pipelined across passes) ---
        def notify_finished(p):
            # p.finish() was just emitted: queue the normalize of its
            # head-quad once both member passes are done, and the
            # out-projection once the whole q-block is done.
            if p.hp % 2 == 1:
                filler.appendleft(normalize(p.qq, p.hp // 2))
            if p.hp == EO - 1:
                filler.append(out_proj(p.qq))

        prev = None
        for qq in range(NQQ):
            if qq + 1 < NQQ:
                queue_setup(qq + 1)
            for hp in range(EO):
                if qq == 0 and hp == 0:
                    p00.steps(KO, do_drain=False)
                    prev = p00
                    continue
                while (qq, hp) not in q_done or qq not in mask_tiles:
                    drain(1)
                ap = AttnPass(qq, hp)
                ap.steps(SKEW, do_drain=False)
                if prev is not None:
                    prev.finish()
                    notify_finished(prev)
                ap.steps(KO)
                prev = ap
        prev.finish()
        notify_finished(prev)
        while filler:
            drain(1)

    nc.compile()
    return nc


def get_program():
    if "nc" not in _prog_cache:
        _prog_cache["nc"] = _build_program()
    return _prog_cache["nc"]


def make_in_maps(K, Q, V, mask, WQ, WK, WV, WO_w, WO_b):
    bf = ml_dtypes.bfloat16
    K = np.asarray(K, dtype=np.float32)
    Q = np.asarray(Q, dtype=np.float32)
    V = np.asarray(V, dtype=np.float32)
    mask = np.asarray(mask)
    WQ = np.asarray(WQ, dtype=np.float32)
    WK = np.asarray(WK, dtype=np.float32)
    WV = np.asarray(WV, dtype=np.float32)
    woT = np.asarray(WO_w, dtype=np.float32).T  # (E, DMODEL)

    qT_b = [np.ascontiguousarray(Q[n].T) for n in range(N)]
    kT_b = [np.ascontiguousarray(K[n].T) for n in range(N)]
    vT_b = [np.ascontiguousarray(V[n].T).astype(bf) for n in range(N)]
    maskT_b = [np.ascontiguousarray(mask[n, 0].T).astype(bf) for n in range(N)]

    in_maps = []
    for c in range(NCORES):
        n, hh = c // 2, c % 2
        hs = slice(hh * HPC, (hh + 1) * HPC)
        # head-concat weight slices: (HPC, D, DK) -> (D, HPC*DK)
        wq_h = np.ascontiguousarray(WQ[hs].transpose(1, 0, 2).reshape(DMODEL, EH))
        wk_h = np.ascontiguousarray(WK[hs].transpose(1, 0, 2).reshape(DMODEL, EH))
        wv_h = np.ascontiguousarray(
            WV[hs].transpose(1, 0, 2).reshape(DMODEL, EH)
        ).astype(bf)
        wo_h = np.ascontiguousarray(woT[hh * EH : (hh + 1) * EH, :]).astype(bf)
        in_maps.append(
            {
                "qT": qT_b[n],
                "kT": kT_b[n],
                "vT": vT_b[n],
                "wq": wq_h,
                "wk": wk_h,
                "wv": wv_h,
                "wo": wo_h,
                "maskT": maskT_b[n],
            }
        )
    return in_maps


def kernel(K, Q, V, mask, WQ, WK, WV, WO_w, WO_b):
    from concourse import bass_utils

    nc = get_program()
    in_maps = make_in_maps(K, Q, V, mask, WQ, WK, WV, WO_w, WO_b)
    res = bass_utils.run_bass_kernel_spmd(
        nc, in_maps, core_ids=list(range(NCORES)), trace=False
    )
    bias = np.asarray(WO_b, dtype=np.float32).reshape(1, DMODEL)
    out = np.empty((N, QLEN, DMODEL), dtype=np.float32)
    for n in range(N):
        out[n] = (
            res.results[2 * n]["out"].astype(np.float32)
            + res.results[2 * n + 1]["out"].astype(np.float32)
            + bias
        )
    return out


# revision 37
# speedup vs baseline: 1.1183x; 1.1183x over previous
# Multi-head attention (N=4, L=2048, D=1024, H=16, DK=64) on 8 NeuronCores.
#
# Sharding: batch x head-half tensor-parallel. Core c = (n, hh) computes the
# full 2048-q attention of batch n for heads [hh*8, hh*8+8), then the partial
# output projection over its 512 e-dims (WO row-sharded). The host sums the
# two partials per batch and adds the bias. No K/V projection is duplicated
# anywhere (pure DP over (n, q-half) computes each batch's K/V proj twice).
#
# Per-core pipeline, engine-balanced (ScalarE exp is the floor: 33.5M
# elements at 1 elem/lane/cycle @1.2GHz):
#   PE     : V/K/Q projections, S^T = KiT^T QiT (contract 64), PV (contract
#            128, M=65 with a ones column for row sums), partial out-proj.
#   ScalarE: exp(S/8) [128,1024] psum->sbuf bf16; V/K evacuations during the
#            startup era before exp begins.
#   DVE    : mask multiply (one broadcast op per k-tile), unnormalized pv
#            evacuation, lazy batched reciprocals, normalize muls, Q evacs.
#   GpSimd : mask multiply for every 4th k-tile (offloads DVE).
#   DMA    : row-sum spill, partition-broadcast of reciprocals (dram round
#            trip), output stores.
# The first attention pass streams directly behind the K projection inside
# the projection era; later passes interleave with remaining projections,
# normalizes and out-proj chunks through a filler queue. PSUM: st 2x2 banks
# + pv 2 + proj/out 2 = 8.
import sys

sys.path.insert(0, "/opt/trn_rl_repo")

import collections
from contextlib import ExitStack

import numpy as np
import ml_dtypes

N, QLEN, KLEN, DMODEL, NHEAD, DK = 4, 2048, 2048, 1024, 16, 64
NCORES = 8
P = 128
HPC = NHEAD // 2  # 8 heads per core
EH = HPC * DK  # 512 e-dims per core
EO = EH // P  # 4 e-tiles (= head pairs)
DO = DMODEL // P  # 8 d-tiles
KO = KLEN // P  # 16 k-tiles
NQQ = 4  # attention q-blocks
QQ = QLEN // NQQ  # 512 q per block
SKEW = 2  # PV trails S/exp/mask by this many k-tiles

_prog_cache = {}


def _build_program():
    import concourse.tile as tile
    from concourse import bacc, mybir

    f32 = mybir.dt.float32
    f32r = mybir.dt.float32r
    bf16 = mybir.dt.bfloat16
    Exp = mybir.ActivationFunctionType.Exp

    nc = bacc.Bacc("TRN2", target_bir_lowering=False, debug=False)

    qT = nc.dram_tensor("qT", (DMODEL, QLEN), f32r, kind="ExternalInput").ap()
    kT = nc.dram_tensor("kT", (DMODEL, KLEN), f32r, kind="ExternalInput").ap()
    vT = nc.dram_tensor("vT", (DMODEL, KLEN), bf16, kind="ExternalInput").ap()
    maskT = nc.dram_tensor("maskT", (KLEN, QLEN), bf16, kind="ExternalInput").ap()
    wq = nc.dram_tensor("wq", (DMODEL, EH), f32r, kind="ExternalInput").ap()
    wk = nc.dram_tensor("wk", (DMODEL, EH), f32r, kind="ExternalInput").ap()
    wv = nc.dram_tensor("wv", (DMODEL, EH), bf16, kind="ExternalInput").ap()
    wo = nc.dram_tensor("wo", (EH, DMODEL), bf16, kind="ExternalInput").ap()
    out = nc.dram_tensor("out", (QLEN, DMODEL), bf16, kind="ExternalOutput").ap()
    rs_dram = nc.dram_tensor("rs_scratch", (NQQ, HPC, QQ), bf16).ap()
    ri_dram = nc.dram_tensor("ri_scratch", (NQQ, HPC, QQ), bf16).ap()

    qT_r = qT.rearrange("(do p) q -> p do q", p=P)
    kT_r = kT.rearrange("(do p) k -> p do k", p=P)
    vT_r = vT.rearrange("(do p) k -> p do k", p=P)
    wq_r = wq.rearrange("(do p) e -> p do e", p=P)
    wk_r = wk.rearrange("(do p) e -> p do e", p=P)
    wv_r = wv.rearrange("(do p) e -> p do e", p=P)
    wo_r = wo.rearrange("(eo p) d -> p eo d", p=P)
    maskT_r = maskT.rearrange("(ko p) q -> p ko q", p=P)

    with tile.TileContext(nc) as tc, ExitStack() as top:
        res = top.enter_context(tc.tile_pool(name="res", bufs=1))
        QiT_s = res.tile([P, EO, QLEN], bf16)  # e=eo*128+p, eo==head pair
        KiT_s = res.tile([P, EO, KLEN], bf16)
        Vi_s = res.tile([P, KO, HPC * 65], bf16)  # k=ko*128+p; col h*65+64 = 1
        headiT_s = res.tile([P, EO, QLEN], bf16)
        wo_s = res.tile([P, EO, DMODEL], bf16)

        gps = top.enter_context(tc.tile_pool(name="gpsum", bufs=2, space="PSUM"))
        sps = top.enter_context(tc.tile_pool(name="spsum", bufs=2, space="PSUM"))
        pvs = top.enter_context(tc.tile_pool(name="pvsum", bufs=1, space="PSUM"))
        ptp = top.enter_context(tc.tile_pool(name="ptile", bufs=SKEW + 4))
        erl = top.enter_context(tc.tile_pool(name="early", bufs=1))

        mask_tiles = {}
        q_stripes = {}
        wq_tiles = {}
        q_psum = {}
        q_done = set()
        filler = collections.deque()

        def drain(n):
            for _ in range(n):
                if not filler:
                    return
                item = filler.popleft()
                if callable(item):
                    item()
                else:  # generator: run one step, put its next step up front
                    try:
                        next(item)
                        filler.appendleft(item)
                    except StopIteration:
                        pass

        def early_loads():
            # q-block-0 inputs, the Q weights (loaded once, shared by every
            # q-block), and the first half of the q-block-0 mask
            qs0 = erl.tile([P, DO, QQ], f32r, tag="qT0")
            for do in range(DO):
                nc.sync.dma_start(qs0[:, do], qT_r[:, do, 0:QQ])
            wcol = erl.tile([P, DO, P], f32r, tag="wq0", name="wq0")
            nc.sync.dma_start(wcol[:], wq_r[:, :, 0:P])
            wq_tiles[0] = wcol
            mk0 = erl.tile([P, KO, QQ], bf16, tag="mask0")
            for ko in range(KO):
                nc.sync.dma_start(mk0[:, ko], maskT_r[:, ko, 0:QQ])
            q_stripes[0] = qs0
            mask_tiles[0] = mk0

        def q_proj(qq, eo, halves=(0, 1)):
            if 0 in halves:
                self_pt = gps.tile([P, 512], f32, tag="gps", name=f"psq{qq}_{eo}")
                q_psum[(qq, eo)] = self_pt
            pt = q_psum[(qq, eo)]
            wcol = wq_tiles[eo]
            qs = q_stripes[qq]
            for h in halves:
                for do in range(h * DO // 2, (h + 1) * DO // 2):
                    nc.tensor.matmul(
                        pt[:],
                        lhsT=wcol[:, do],
                        rhs=qs[:, do],
                        start=(do == 0),
                        stop=(do == DO - 1),
                    )
            if 1 in halves:
                nc.vector.tensor_copy(
                    out=QiT_s[:, eo, qq * QQ : (qq + 1) * QQ], in_=pt[:]
                )
                del q_psum[(qq, eo)]
                q_done.add((qq, eo))

        class AttnPass:
            def __init__(self, qq, hp):
                self.qq, self.hp = qq, hp
                self.mk = mask_tiles[qq]
                self.pv = None
                self.ptq = {}
                self.next_ko = 0

            def emit_pv(self, ko):
                if self.pv is None:
                    # allocated lazily so the pool sees this AFTER the
                    # previous pass's trailing reads (cross-pass pipelining)
                    self.pv = [
                        pvs.tile(
                            [65, QQ], f32, tag=f"pv{i}",
                            name=f"pv{i}_{self.qq}_{self.hp}",
                        )
                        for i in range(2)
                    ]
                pt = self.ptq.pop(ko)
                hp = self.hp
                for i in range(2):
                    nc.tensor.matmul(
                        self.pv[i][:],
                        lhsT=Vi_s[:, ko, (2 * hp + i) * 65 : (2 * hp + i + 1) * 65],
                        rhs=pt[:, i * QQ : (i + 1) * QQ],
                        start=(ko == 0),
                        stop=(ko == KO - 1),
                        skip_group_check=True,
                    )

            def steps(self, ko_end, do_drain=True):
                qq, hp = self.qq, self.hp
                for ko in range(self.next_ko, ko_end):
                    if ko >= SKEW:
                        self.emit_pv(ko - SKEW)
                    st = sps.tile([P, 2 * QQ], f32, tag="st", name=f"st{qq}_{hp}_{ko}")
                    for i in range(2):
                        p0 = 64 * i
                        nc.tensor.matmul(
                            st[:, i * QQ : (i + 1) * QQ],
                            lhsT=KiT_s[p0 : p0 + 64, hp, ko * P : (ko + 1) * P],
                            rhs=QiT_s[p0 : p0 + 64, hp, qq * QQ : (qq + 1) * QQ],
                            start=True,
                            stop=True,
                        )
                    pt = ptp.tile([P, 2 * QQ], bf16, tag="pt", name=f"pt{qq}_{hp}_{ko}")
                    nc.scalar.activation(out=pt[:], in_=st[:], func=Exp, scale=0.125)
                    if False:  # GpSimd mask share disabled (jitter)
                        for i in range(2):
                            nc.gpsimd.tensor_mul(
                                out=pt[:, i * QQ : (i + 1) * QQ],
                                in0=pt[:, i * QQ : (i + 1) * QQ],
                                in1=self.mk[:, ko],
                            )
                    else:
                        nc.vector.tensor_mul(
                            out=pt[:].rearrange("p (i q) -> p i q", i=2),
                            in0=pt[:].rearrange("p (i q) -> p i q", i=2),
                            in1=self.mk[:, ko, None, :].to_broadcast([P, 2, QQ]),
                        )
                    self.ptq[ko] = pt
                    if do_drain and ko % 2 == 1:
                        drain(1)
                self.next_ko = ko_end

            def finish(self):
                qq, hp = self.qq, self.hp
                for ko in range(KO - SKEW, KO):
                    self.emit_pv(ko)
                # evacuate unnormalized heads; spill row sums for the lazy
                # batched normalize.
                rssb = rsp.tile([1, 2, QQ], bf16, tag="rssb", name=f"rssb{qq}_{hp}")
                for i in range(2):
                    nc.vector.tensor_copy(
                        out=headiT_s[64 * i : 64 * i + 64, hp, qq * QQ : (qq + 1) * QQ],
                        in_=self.pv[i][0:64, :],
                    )
                    nc.scalar.copy(out=rssb[:, i, :], in_=self.pv[i][64:65, :])
                nc.sync.dma_start(rs_dram[qq, 2 * hp : 2 * hp + 2], rssb[0:1])

        # ---------- projection era: V + K stripes, pass (0,0) streamed ----
        p00 = None
        early_loads()
        with ExitStack() as ph:
            vsp = ph.enter_context(tc.tile_pool(name="vstripe", bufs=2))
            ksp = ph.enter_context(tc.tile_pool(name="kstripe", bufs=2))
            wvp = ph.enter_context(tc.tile_pool(name="wvres", bufs=1))
            wv_s = wvp.tile([P, DO, EH], bf16)
            for do in range(DO):
                nc.sync.dma_start(wv_s[:, do], wv_r[:, do])
            wkp = ph.enter_context(tc.tile_pool(name="wkp", bufs=1))
            wks = {}
            for eo in range(EO):
                wcol = wkp.tile([P, DO, P], f32r, tag=f"wk{eo}", name=f"wk{eo}")
                nc.sync.dma_start(wcol[:], wk_r[:, :, eo * P : (eo + 1) * P])
                wks[eo] = wcol
            nc.vector.memset(Vi_s[:, :, 64::65], 1.0)
            for s in range(4):  # 512-column stripes of kT; 2x256 of vT
                for half in range(2):
                    vss = 2 * s + half
                    vs = vsp.tile([P, DO, 256], bf16, tag="v", name=f"vT{vss}")
                    for do in range(DO):
                        nc.sync.dma_start(
                            vs[:, do], vT_r[:, do, vss * 256 : (vss + 1) * 256]
                        )
                    for t in range(2):
                        ko = vss * 2 + t
                        pt = gps.tile([P, 512], f32, tag="gps", name=f"psv{ko}")
                        for do in range(DO):
                            nc.tensor.matmul(
                                pt[:],
                                lhsT=vs[:, do, t * P : (t + 1) * P],
                                rhs=wv_s[:, do],
                                start=(do == 0),
                                stop=(do == DO - 1),
                            )
                        nc.scalar.copy(
                            out=Vi_s[:, ko, :].rearrange("p (h j) -> p h j", j=65)[
                                :, :, 0:64
                            ],
                            in_=pt[:].rearrange("p (h j) -> p h j", j=64),
                        )
                ks = ksp.tile([P, DO, 512], f32r, tag="k", name=f"kT{s}")
                for do in range(DO):
                    nc.sync.dma_start(ks[:, do], kT_r[:, do, s * 512 : (s + 1) * 512])
                for eo in range(EO):
                    pt = gps.tile([P, 512], f32, tag="gps", name=f"psk{s}_{eo}")
                    for do in range(DO):
                        nc.tensor.matmul(
                            pt[:],
                            lhsT=wks[eo][:, do],
                            rhs=ks[:, do],
                            start=(do == 0),
                            stop=(do == DO - 1),
                        )
                    if eo % 2 == 0:
                        nc.scalar.copy(
                            out=KiT_s[:, eo, s * 512 : (s + 1) * 512], in_=pt[:]
                        )
                    else:
                        nc.vector.tensor_copy(
                            out=KiT_s[:, eo, s * 512 : (s + 1) * 512], in_=pt[:]
                        )
                # stream the first attention pass right behind the K proj
                if s == 0:
                    q_proj(0, 0)
                    p00 = AttnPass(0, 0)
                    continue
                if s == 1:
                    for eo in range(1, EO):
                        wcol = erl.tile(
                            [P, DO, P], f32r, tag=f"wq{eo}", name=f"wq{eo}"
                        )
                        nc.sync.dma_start(wcol[:], wq_r[:, :, eo * P : (eo + 1) * P])
                        wq_tiles[eo] = wcol
                if s == 3:
                    for eo in range(EO):
                        nc.sync.dma_start(wo_s[:, eo], wo_r[:, eo])
                p00.steps(min(4 * s - 2, KO), do_drain=(s >= 2))

        mkp = top.enter_context(tc.tile_pool(name="mask", bufs=2))
        qsp = top.enter_context(tc.tile_pool(name="qstripe", bufs=2))
        nrm = top.enter_context(tc.tile_pool(name="norm", bufs=2))
        rsp = top.enter_context(tc.tile_pool(name="rsp", bufs=1))

        def load_mask(qq):
            mk = mkp.tile([P, KO, QQ], bf16, tag="mask", name=f"mask{qq}")
            for ko in range(KO):
                nc.sync.dma_start(mk[:, ko], maskT_r[:, ko, qq * QQ : (qq + 1) * QQ])
            mask_tiles[qq] = mk

        def load_qstripe(qq):
            qs = qsp.tile([P, DO, QQ], f32r, tag="qT", name=f"qT{qq}")
            for do in range(DO):
                nc.sync.dma_start(qs[:, do], qT_r[:, do, qq * QQ : (qq + 1) * QQ])
            q_stripes[qq] = qs

        def normalize(qq, hq):
            # 1/rowsum for one head-quad: rowsums were spilled to rs_dram by
            # the two finished passes; DVE reciprocal, spill back, then
            # partition-broadcast DMAs feed the normalize multiply.
            sl = slice(4 * hq, 4 * hq + 4)
            rsall = rsp.tile([4, QQ], bf16, tag="rsall", name=f"rsa{qq}_{hq}")
            nc.sync.dma_start(rsall[:], rs_dram[qq, sl])
            with nc.allow_low_precision(reason="softmax reciprocal in bf16"):
                nc.vector.reciprocal(out=rsall[:], in_=rsall[:])
            yield
            nc.sync.dma_start(ri_dram[qq, sl], rsall[:])
            rr = rsp.tile([P, 2, QQ], bf16, tag="rr", name=f"rr{qq}_{hq}")
            for j in range(2):
                for i in range(2):
                    nc.sync.dma_start(
                        rr[64 * i : 64 * i + 64, j, :],
                        ri_dram[qq, 2 * (2 * hq + j) + i].partition_broadcast(64),
                    )
            yield
            nc.vector.tensor_mul(
                out=headiT_s[:, 2 * hq : 2 * hq + 2, qq * QQ : (qq + 1) * QQ],
                in0=headiT_s[:, 2 * hq : 2 * hq + 2, qq * QQ : (qq + 1) * QQ],
                in1=rr[:],
            )
            yield

        def out_proj(qq):
            for t in range(QQ // P):  # 4 q-tiles of 128
                qt = qq * (QQ // P) + t
                for dc in range(2):
                    pt = gps.tile([P, 512], f32, tag="gps", name=f"pso{qt}_{dc}")
                    for eo in range(EO):
                        nc.tensor.matmul(
                            pt[:],
                            lhsT=headiT_s[:, eo, qt * P : (qt + 1) * P],
                            rhs=wo_s[:, eo, dc * 512 : (dc + 1) * 512],
                            start=(eo == 0),
                            stop=(eo == EO - 1),
                        )
                    ot = nrm.tile([P, 512], bf16, tag="ot", name=f"o{qt}_{dc}")
                    nc.vector.tensor_copy(out=ot[:], in_=pt[:])
                    nc.sync.dma_start(
                        out[qt * P : (qt + 1) * P, dc * 512 : (dc + 1) * 512],
                        ot[:],
                    )
                    yield

        for eo in range(1, EO):
            filler.append(lambda eo=eo: q_proj(0, eo, (0,)))
            filler.append(lambda eo=eo: q_proj(0, eo, (1,)))

        def queue_setup(qq):
            filler.append(lambda: (load_mask(qq), load_qstripe(qq)))
            for eo in range(EO):
                filler.append(lambda eo=eo: q_proj(qq, eo, (0,)))
                filler.append(lambda eo=eo: q_proj(qq, eo, (1,)))

        # ---------- attention passes (software-pipelined across passes) ---
        def notify_finished(p):
            # p.finish() was just emitted: queue the normalize of its
            # head-quad once both member passes are done, and the
            # out-projection once the whole q-block is done.
            if p.hp % 2 == 1:
                filler.appendleft(normalize(p.qq, p.hp // 2))
            if p.hp == EO - 1:
                filler.append(out_proj(p.qq))

        prev = None
        for qq in range(NQQ):
            if qq + 1 < NQQ:
                queue_setup(qq + 1)
            for hp in range(EO):
                if qq == 0 and hp == 0:
                    p00.steps(KO, do_drain=False)
                    prev = p00
                    continue
                while (qq, hp) not in q_done or qq not in mask_tiles:
                    drain(1)
                ap = AttnPass(qq, hp)
                ap.steps(SKEW, do_drain=False)
                if prev is not None:
                    prev.finish()
                    notify_finished(prev)
                ap.steps(KO)
                prev = ap
        prev.finish()
        notify_finished(prev)
        while filler:
            drain(1)

    nc.compile()
    return nc


def get_program():
    if "nc" not in _prog_cache:
        _prog_cache["nc"] = _build_program()
    return _prog_cache["nc"]


def make_in_maps(K, Q, V, mask, WQ, WK, WV, WO_w, WO_b):
    bf = ml_dtypes.bfloat16
    K = np.asarray(K, dtype=np.float32)
    Q = np.asarray(Q, dtype=np.float32)
    V = np.asarray(V, dtype=np.float32)
    mask = np.asarray(mask)
    WQ = np.asarray(WQ, dtype=np.float32)
    WK = np.asarray(WK, dtype=np.float32)
    WV = np.asarray(WV, dtype=np.float32)
    woT = np.asarray(WO_w, dtype=np.float32).T  # (E, DMODEL)

    qT_b = [np.ascontiguousarray(Q[n].T) for n in range(N)]
    kT_b = [np.ascontiguousarray(K[n].T) for n in range(N)]
    vT_b = [np.ascontiguousarray(V[n].T).astype(bf) for n in range(N)]
    maskT_b = [np.ascontiguousarray(mask[n, 0].T).astype(bf) for n in range(N)]

    in_maps = []
    for c in range(NCORES):
        n, hh = c // 2, c % 2
        hs = slice(hh * HPC, (hh + 1) * HPC)
        # head-concat weight slices: (HPC, D, DK) -> (D, HPC*DK)
        wq_h = np.ascontiguousarray(WQ[hs].transpose(1, 0, 2).reshape(DMODEL, EH))
        wk_h = np.ascontiguousarray(WK[hs].transpose(1, 0, 2).reshape(DMODEL, EH))
        wv_h = np.ascontiguousarray(
            WV[hs].transpose(1, 0, 2).reshape(DMODEL, EH)
        ).astype(bf)
        wo_h = np.ascontiguousarray(woT[hh * EH : (hh + 1) * EH, :]).astype(bf)
        in_maps.append(
            {
                "qT": qT_b[n],
                "kT": kT_b[n],
                "vT": vT_b[n],
                "wq": wq_h,
                "wk": wk_h,
                "wv": wv_h,
                "wo": wo_h,
                "maskT": maskT_b[n],
            }
        )
    return in_maps


def kernel(K, Q, V, mask, WQ, WK, WV, WO_w, WO_b):
    from concourse import bass_utils

    nc = get_program()
    in_maps = make_in_maps(K, Q, V, mask, WQ, WK, WV, WO_w, WO_b)
    res = bass_utils.run_bass_kernel_spmd(
        nc, in_maps, core_ids=list(range(NCORES)), trace=False
    )
    bias = np.asarray(WO_b, dtype=np.float32).reshape(1, DMODEL)
    out = np.empty((N, QLEN, DMODEL), dtype=np.float32)
    for n in range(N):
        out[n] = (
            res.results[2 * n]["out"].astype(np.float32)
            + res.results[2 * n + 1]["out"].astype(np.float32)
            + bias
        )
    return out
